# revision 10
# baseline (speedup 1.0000x reference)
"""Binary-weight 3x3 conv via 1D Winograd F(2,3) along H on 8 TRN2 cores.

Data-parallel over batch (4 images/core). The H-axis 3-tap conv is
Winograd-transformed: the host computes V[u] = B^T d row combos (u=0..3,
28 tile-rows of 2 output rows each) and ships them as bf16 in the same
pitch-57 padded layout the direct kernel used (so the W-axis stays a
direct conv via shifted contiguous windows). Weights become GW = G.sign(W)
per (u, kw) -- values {0, +-1/2, +-1, +-3/2}, exact in bf16. The PE
accumulates M[u] = sum_{cit,kw} GW[u,kw]^T V[u] (6-matmul chains, N=456)
into 4 PSUM banks per 8-ty-row block; the A^T recombination
(even rows = M0+M1+M2, odd = M1-M2-M3) runs on the otherwise-idle vector
engine. PE work drops 1.5x vs the direct kernel (24 matmuls of 456/228
per 16 output rows vs 36): ~131us streaming floor vs 191us.

Host V transform in f32, cast to bf16: measured 2.1e-3 end-to-end rel
err (budget 2e-2).
"""

import numpy as np
import ml_dtypes

N_CORES = 8
B_PER_CORE = 4
H = W = 56
VP = 57  # pitch: 56 data cols + 1 shared pad col
VROWS = 28  # ty tile-rows (2 output rows each)
VLEN = 1 + VROWS * VP + 1  # 1598
BLOCKS = [(0, 8), (8, 8), (16, 8), (24, 4)]  # (ty0, tn)

_CACHED = {}


def _build_nc():
    import concourse.mybir as mybir
    from concourse import bacc
    from concourse.tile import TileContext

    f32 = mybir.dt.float32
    bf16 = mybir.dt.bfloat16
    ADD = mybir.AluOpType.add
    SUB = mybir.AluOpType.subtract

    nc = bacc.Bacc("TRN2", target_bir_lowering=False, debug=False)
    xs = nc.dram_tensor(
        "xs", [B_PER_CORE, 2, 4, 128, VLEN], bf16, kind="ExternalInput"
    ).ap()
    wt = nc.dram_tensor(
        "wt", [2, 2, 128, 4, 3, 128], bf16, kind="ExternalInput"
    ).ap()
    out = nc.dram_tensor(
        "out", [B_PER_CORE, 256, H, W], f32, kind="ExternalOutput"
    ).ap()

    with TileContext(nc) as tc:
        with (
            tc.tile_pool(name="wp", bufs=1) as wp,
            tc.tile_pool(name="xp", bufs=8) as xp,
            tc.tile_pool(name="yp", bufs=2) as yp,
            tc.tile_pool(name="tp", bufs=8) as tp,
            tc.tile_pool(name="pp", bufs=8, space="PSUM") as pp,
        ):
            # [128ci, ct, cit, u, kw, 128co]
            w_sb = wp.tile([128, 2, 2, 4, 3, 128], bf16, name="w_sb")

            vt = {}
            HV = VLEN // 2

            def load_image(n):
                for cit in range(2):
                    t = xp.tile([128, 4, VLEN], bf16, name=f"v_{n}_{cit}", tag="v")
                    vt[(n, cit)] = t
                    if n == 0 and cit == 0:
                        # first weight block (ct0, cit0) leads on scalar
                        nc.scalar.dma_start(out=w_sb[:, 0, 0], in_=wt[0, 0])
                    if n == 0:
                        # image 0: per-plane halves across both queues
                        for u in range(4):
                            nc.sync.dma_start(
                                out=t[:, u, :HV], in_=xs[n, cit, u, :, :HV]
                            )
                            nc.scalar.dma_start(
                                out=t[:, u, HV:], in_=xs[n, cit, u, :, HV:]
                            )
                        if cit == 0:
                            nc.sync.dma_start(out=w_sb[:, 0, 1], in_=wt[0, 1])
                        else:
                            nc.sync.dma_start(out=w_sb[:, 1, 0], in_=wt[1, 0])
                            nc.scalar.dma_start(out=w_sb[:, 1, 1], in_=wt[1, 1])
                    else:
                        for u in range(4):
                            eng = nc.sync if u % 2 == 0 else nc.scalar
                            eng.dma_start(out=t[:, u], in_=xs[n, cit, u])

            for _n in range(B_PER_CORE):
                load_image(_n)

            def valid(ap, tn):
                # [128, tn*57] -> [128, tn, 56] (drop the junk col per row)
                return ap.rearrange("p (t w) -> p t w", w=VP)[:, :, :W]

            group_i = 0
            n_groups = B_PER_CORE * 2
            for n in range(B_PER_CORE):
                for ct in range(2):
                    last = group_i == n_groups - 1
                    yg = yp.tile([128, H * W], f32, name="yg", tag="yg")
                    for ty0, tn in BLOCKS:
                        NF = tn * VP
                        M = [
                            pp.tile([128, 512], f32, name=f"m{u}", tag="ps")
                            for u in range(4)
                        ]
                        for cit in range(2):
                            for u in range(4):
                                for kw in range(3):
                                    nc.tensor.matmul(
                                        M[u][:, :NF],
                                        lhsT=w_sb[:, ct, cit, u, kw, :],
                                        rhs=vt[(n, cit)][
                                            :, u, ty0 * VP + kw : ty0 * VP + kw + NF
                                        ],
                                        start=(cit == 0 and kw == 0),
                                        stop=(cit == 1 and kw == 2),
                                    )
                        # A^T on DVE: even = M0+M1+M2, odd = M1-M2-M3.
                        # tensor_tensor can't take two PSUM operands, so M1/M2
                        # come down to SBUF first.
                        c1 = tp.tile([128, 456], f32, name="c1", tag="c1")
                        c2 = tp.tile([128, 456], f32, name="c2", tag="c2")
                        nc.vector.tensor_copy(out=c1[:, :NF], in_=M[1][:, :NF])
                        nc.vector.tensor_copy(out=c2[:, :NF], in_=M[2][:, :NF])
                        te = tp.tile([128, 456], f32, name="te", tag="te")
                        to = tp.tile([128, 456], f32, name="to", tag="to")
                        nc.vector.tensor_tensor(
                            out=te[:, :NF], in0=c1[:, :NF], in1=c2[:, :NF], op=ADD
                        )
                        nc.vector.tensor_tensor(
                            out=to[:, :NF], in0=c1[:, :NF], in1=c2[:, :NF], op=SUB
                        )
                        rows = yg[:, 2 * ty0 * W : 2 * (ty0 + tn) * W].rearrange(
                            "p (t two w) -> p t two w", two=2, w=W
                        )
                        nc.vector.tensor_tensor(
                            out=rows[:, :, 0],
                            in0=M[0][:, :NF].rearrange("p (t w) -> p t w", w=VP)[
                                :, :, :W
                            ],
                            in1=valid(te[:, :NF], tn),
                            op=ADD,
                        )
                        nc.vector.tensor_tensor(
                            out=rows[:, :, 1],
                            in0=valid(to[:, :NF], tn),
                            in1=M[3][:, :NF].rearrange("p (t w) -> p t w", w=VP)[
                                :, :, :W
                            ],
                            op=SUB,
                        )
                        if last:
                            # per-block DMAs keep the tail short
                            eng = nc.sync if (ty0 // 8) % 2 == 0 else nc.scalar
                            eng.dma_start(
                                out=out[
                                    n,
                                    ct * 128 : (ct + 1) * 128,
                                    2 * ty0 : 2 * (ty0 + tn),
                                    :,
                                ],
                                in_=yg[:, 2 * ty0 * W : 2 * (ty0 + tn) * W],
                            )
                    if not last:
                        eng = nc.sync if group_i % 2 == 0 else nc.scalar
                        eng.dma_start(
                            out=out[n, ct * 128 : (ct + 1) * 128], in_=yg[:]
                        )
                    group_i += 1
    nc.compile()
    return nc


def _get_nc():
    if "nc" not in _CACHED:
        _CACHED["nc"] = _build_nc()
    return _CACHED["nc"]


_G = np.array(
    [[1, 0, 0], [0.5, 0.5, 0.5], [0.5, -0.5, 0.5], [0, 0, 1]], np.float32
)


def _prep_weights(W_arr):
    Wb = np.sign(np.asarray(W_arr, dtype=np.float32))
    wt = np.zeros((2, 2, 128, 4, 3, 128), np.float32)
    for ct in range(2):
        for cit in range(2):
            blk = Wb[ct * 128 : (ct + 1) * 128, cit * 128 : (cit + 1) * 128]
            # [u,kw,ci,co] -> [ci,u,kw,co]
            gw = np.einsum("uh,oihw->uwio", _G, blk)
            wt[ct, cit] = gw.transpose(2, 0, 1, 3)
    return np.ascontiguousarray(wt).astype(ml_dtypes.bfloat16)


def _prep_x(x):
    """Host B^T row transform -> V[u] planes in the padded pitch-57 layout."""
    x = np.asarray(x, dtype=np.float32)
    B = x.shape[0]
    xr = x.reshape(B, 2, 128, 56, 56)
    D = np.zeros((B, 2, 128, 58, 56), np.float32)
    D[..., 1:57, :] = xr
    r0 = D[..., 0:56:2, :]
    r1 = D[..., 1:57:2, :]
    r2 = D[..., 2:58:2, :]
    r3 = D[..., 3:58:2, :]
    V = np.stack([r0 - r2, r1 + r2, r2 - r1, r1 - r3], axis=2)  # [B,2,4,128,28,56]
    vflat = np.zeros((B, 2, 4, 128, VLEN), dtype=ml_dtypes.bfloat16)
    vv = vflat[..., 1 : 1 + VROWS * VP].reshape(B, 2, 4, 128, VROWS, VP)
    vv[..., :W] = V.astype(ml_dtypes.bfloat16)
    return vflat


def run(x, W, trace=False, trace_kwargs=None):
    from concourse.bass_utils import run_bass_kernel_spmd

    xp = _prep_x(x)
    wt = _prep_weights(W)
    nc = _get_nc()
    in_maps = [
        {
            "xs": np.ascontiguousarray(xp[i * B_PER_CORE : (i + 1) * B_PER_CORE]),
            "wt": wt,
        }
        for i in range(N_CORES)
    ]
    res = run_bass_kernel_spmd(
        nc,
        in_maps,
        list(range(N_CORES)),
        trace=trace,
        trace_kwargs=trace_kwargs or {},
    )
    out = np.concatenate([np.asarray(res.results[i]["out"]) for i in range(N_CORES)])
    return out, res


def kernel(x, W):
    out, _ = run(x, W, trace=False)
    return out


# revision 13
# speedup vs baseline: 1.1983x; 1.1983x over previous
"""Binary-weight 3x3 conv via 1D Winograd F(4,3) along H on 8 TRN2 cores.

Like the F(2,3) version but with 4 output rows per tile-row: the host
ships V[u] = B^T d (u=0..5, 14 tile-rows) in the pitch-57 padded layout;
weights are GW = G.sign(W) in bf16 (G has 1/6, 1/24 entries -- inexact,
measured 7.3e-3 end-to-end vs the 2e-2 budget). The PE accumulates six
M[u] PSUM banks per 7-ty-row block (6-matmul chains, N=399); H-axis PE
work drops 2x vs direct (36 matmuls of 399 per 28 output rows).

A^T stage per block, ordered to free PSUM banks in pool order (6 banks +
2 spare means the next block's chains need them back promptly): copy
M0..M5 -> c0..c5 (SBUF), then s=c1+c2, dd=c1-c2, t=c3+c4, dt=c3-c4,
y0=c0+s+t, y1=dd+2dt, y2=s+4t, y3=dd+8dt+c5 (scaled adds via
scalar_tensor_tensor), ~5.2us DVE vs 6.0us PE per block.
"""

import numpy as np
import ml_dtypes

N_CORES = 8
B_PER_CORE = 4
H = W = 56
VP = 57
VROWS = 14  # ty tile-rows (4 output rows each)
VLEN = 1 + VROWS * VP + 1  # 800
BLOCKS = [(0, 7), (7, 7)]  # (ty0, tn), N = 399

_CACHED = {}


def _build_nc():
    import concourse.mybir as mybir
    from concourse import bacc
    from concourse.tile import TileContext

    f32 = mybir.dt.float32
    bf16 = mybir.dt.bfloat16
    ADD = mybir.AluOpType.add
    SUB = mybir.AluOpType.subtract
    MUL = mybir.AluOpType.mult

    nc = bacc.Bacc("TRN2", target_bir_lowering=False, debug=False)
    xs = nc.dram_tensor(
        "xs", [B_PER_CORE, 2, 128, 6, VLEN], bf16, kind="ExternalInput"
    ).ap()
    wt = nc.dram_tensor(
        "wt", [2, 2, 128, 6, 3, 128], bf16, kind="ExternalInput"
    ).ap()
    out = nc.dram_tensor(
        "out", [B_PER_CORE, 256, H, W], f32, kind="ExternalOutput"
    ).ap()

    with TileContext(nc) as tc:
        with (
            tc.tile_pool(name="wp", bufs=1) as wp,
            tc.tile_pool(name="xp", bufs=8) as xp,
            tc.tile_pool(name="yp", bufs=2) as yp,
            tc.tile_pool(name="tp", bufs=2) as tp,
            tc.tile_pool(name="pp", bufs=8, space="PSUM") as pp,
        ):
            # [128ci, ct, cit, u, kw, 128co]
            w_sb = wp.tile([128, 2, 2, 6, 3, 128], bf16, name="w_sb")

            vt = {}

            def load_image(n):
                for cit in range(2):
                    t = xp.tile([128, 6, VLEN], bf16, name=f"v_{n}_{cit}", tag="v")
                    vt[(n, cit)] = t
                    if n == 0 and cit == 0:
                        # u0 weight block leads so the first chain unblocks fast
                        nc.scalar.dma_start(
                            out=w_sb[:, 0, 0, 0], in_=wt[0, 0, :, 0]
                        )
                        nc.scalar.dma_start(
                            out=w_sb[:, 0, 0, 1:], in_=wt[0, 0, :, 1:]
                        )
                    if n == 0:
                        for u in range(6):
                            eng = nc.sync if u % 2 == 0 else nc.scalar
                            eng.dma_start(out=t[:, u], in_=xs[n, cit, :, u])
                        if cit == 0:
                            nc.sync.dma_start(out=w_sb[:, 0, 1], in_=wt[0, 1])
                        else:
                            nc.sync.dma_start(out=w_sb[:, 1, 0], in_=wt[1, 0])
                            nc.scalar.dma_start(out=w_sb[:, 1, 1], in_=wt[1, 1])
                    else:
                        nc.sync.dma_start(out=t[:, :3], in_=xs[n, cit, :, :3])
                        nc.scalar.dma_start(out=t[:, 3:], in_=xs[n, cit, :, 3:])

            for _n in range(B_PER_CORE):
                load_image(_n)

            def valid(ap):
                return ap.rearrange("p (t w) -> p t w", w=VP)[:, :, :W]

            group_i = 0
            n_groups = B_PER_CORE * 2
            for n in range(B_PER_CORE):
                for ct in range(2):
                    last = group_i == n_groups - 1
                    yg = yp.tile([128, H * W], f32, name="yg", tag="yg")
                    for ty0, tn in BLOCKS:
                        NF = tn * VP  # 399
                        M = [
                            pp.tile([128, 512], f32, name=f"m{u}", tag="ps")
                            for u in range(6)
                        ]
                        for cit in range(2):
                            for u in range(6):
                                for kw in range(3):
                                    nc.tensor.matmul(
                                        M[u][:, :NF],
                                        lhsT=w_sb[:, ct, cit, u, kw, :],
                                        rhs=vt[(n, cit)][
                                            :, u, ty0 * VP + kw : ty0 * VP + kw + NF
                                        ],
                                        start=(cit == 0 and kw == 0),
                                        stop=(cit == 1 and kw == 2),
                                    )
                        # copies first, in pool order, to free banks promptly;
                        # they run on the otherwise-idle ACT engine so the DVE
                        # only does the 10 combo ops (~5.8us/block < 6us PE)
                        c = []
                        for u in range(6):
                            cu = tp.tile([128, NF], f32, name=f"c{u}", tag=f"c{u}")
                            nc.scalar.activation(
                                out=cu[:],
                                in_=M[u][:, :NF],
                                func=mybir.ActivationFunctionType.Copy,
                            )
                            c.append(cu)
                        s = tp.tile([128, NF], f32, name="s", tag="s")
                        dd = tp.tile([128, NF], f32, name="dd", tag="dd")
                        t4 = tp.tile([128, NF], f32, name="t4", tag="t4")
                        dt = tp.tile([128, NF], f32, name="dt", tag="dt")
                        st = tp.tile([128, NF], f32, name="st", tag="st")
                        t3 = tp.tile([128, NF], f32, name="t3", tag="t3")
                        nc.vector.tensor_tensor(out=s[:], in0=c[1][:], in1=c[2][:], op=ADD)
                        nc.vector.tensor_tensor(out=dd[:], in0=c[1][:], in1=c[2][:], op=SUB)
                        nc.vector.tensor_tensor(out=t4[:], in0=c[3][:], in1=c[4][:], op=ADD)
                        nc.vector.tensor_tensor(out=dt[:], in0=c[3][:], in1=c[4][:], op=SUB)
                        rows = yg[
                            :, 4 * ty0 * W : 4 * (ty0 + tn) * W
                        ].rearrange("p (t four w) -> p t four w", four=4, w=W)
                        # y0 = c0 + s + t
                        nc.vector.tensor_tensor(out=st[:], in0=s[:], in1=t4[:], op=ADD)
                        nc.vector.tensor_tensor(
                            out=rows[:, :, 0], in0=valid(st[:]), in1=valid(c[0][:]), op=ADD
                        )
                        # y1 = 2*dt + dd
                        nc.vector.scalar_tensor_tensor(
                            out=rows[:, :, 1], in0=valid(dt[:]), scalar=2.0,
                            in1=valid(dd[:]), op0=MUL, op1=ADD,
                        )
                        # y2 = 4*t + s
                        nc.vector.scalar_tensor_tensor(
                            out=rows[:, :, 2], in0=valid(t4[:]), scalar=4.0,
                            in1=valid(s[:]), op0=MUL, op1=ADD,
                        )
                        # y3 = 8*dt + dd + c5
                        nc.vector.scalar_tensor_tensor(
                            out=t3[:], in0=dt[:], scalar=8.0,
                            in1=dd[:], op0=MUL, op1=ADD,
                        )
                        nc.vector.tensor_tensor(
                            out=rows[:, :, 3], in0=valid(t3[:]), in1=valid(c[5][:]), op=ADD
                        )
                        if last:
                            eng = nc.sync if (ty0 // 7) % 2 == 0 else nc.scalar
                            eng.dma_start(
                                out=out[
                                    n,
                                    ct * 128 : (ct + 1) * 128,
                                    4 * ty0 : 4 * (ty0 + tn),
                                    :,
                                ],
                                in_=yg[:, 4 * ty0 * W : 4 * (ty0 + tn) * W],
                            )
                    if not last:
                        eng = nc.sync if group_i % 2 == 0 else nc.scalar
                        eng.dma_start(
                            out=out[n, ct * 128 : (ct + 1) * 128], in_=yg[:]
                        )
                    group_i += 1
    nc.compile()
    return nc


def _get_nc():
    if "nc" not in _CACHED:
        _CACHED["nc"] = _build_nc()
    return _CACHED["nc"]


_BT = np.array(
    [
        [4, 0, -5, 0, 1, 0],
        [0, -4, -4, 1, 1, 0],
        [0, 4, -4, -1, 1, 0],
        [0, -2, -1, 2, 1, 0],
        [0, 2, -1, -2, 1, 0],
        [0, 4, 0, -5, 0, 1],
    ],
    np.float32,
)
_G = np.array(
    [
        [1 / 4, 0, 0],
        [-1 / 6, -1 / 6, -1 / 6],
        [-1 / 6, 1 / 6, -1 / 6],
        [1 / 24, 1 / 12, 1 / 6],
        [1 / 24, -1 / 12, 1 / 6],
        [0, 0, 1],
    ],
    np.float32,
)


def _prep_weights(W_arr):
    Wb = np.sign(np.asarray(W_arr, dtype=np.float32))
    wt = np.zeros((2, 2, 128, 6, 3, 128), np.float32)
    for ct in range(2):
        for cit in range(2):
            blk = Wb[ct * 128 : (ct + 1) * 128, cit * 128 : (cit + 1) * 128]
            gw = np.einsum("uh,oihw->uwio", _G, blk)  # [u,kw,ci,co]
            wt[ct, cit] = gw.transpose(2, 0, 1, 3)
    return np.ascontiguousarray(wt).astype(ml_dtypes.bfloat16)


def _prep_x(x):
    x = np.asarray(x, dtype=np.float32)
    B = x.shape[0]
    xr = x.reshape(B, 2, 128, 56, 56)
    D = np.zeros((B, 2, 128, 58, 56), np.float32)
    D[..., 1:57, :] = xr
    Dr = np.stack([D[..., r : r + 53 : 4, :] for r in range(6)], axis=3)
    V = np.einsum("ur,bcirty->bcuity", _BT, Dr)  # [B,2,6,128,14,56]
    vflat = np.zeros((B, 2, 128, 6, VLEN), dtype=ml_dtypes.bfloat16)
    vv = vflat[..., 1 : 1 + VROWS * VP].reshape(B, 2, 128, 6, VROWS, VP)
    vv[..., :W] = V.transpose(0, 1, 3, 2, 4, 5).astype(ml_dtypes.bfloat16)
    return vflat


def run(x, W, trace=False, trace_kwargs=None):
    from concourse.bass_utils import run_bass_kernel_spmd

    xp = _prep_x(x)
    wt = _prep_weights(W)
    nc = _get_nc()
    in_maps = [
        {
            "xs": np.ascontiguousarray(xp[i * B_PER_CORE : (i + 1) * B_PER_CORE]),
            "wt": wt,
        }
        for i in range(N_CORES)
    ]
    res = run_bass_kernel_spmd(
        nc,
        in_maps,
        list(range(N_CORES)),
        trace=trace,
        trace_kwargs=trace_kwargs or {},
    )
    out = np.concatenate([np.asarray(res.results[i]["out"]) for i in range(N_CORES)])
    return out, res


def kernel(x, W):
    out, _ = run(x, W, trace=False)
    return out
